# revision 32
# baseline (speedup 1.0000x reference)
"""BankModulatedConv Trainium2 kernel.

Problem (per sample b of B=8, one NeuronCore per sample):
  w = softmax(bank_request[b])                        # (16,)
  kern = sum_f w[f] * bank_weight[f]                  # (o, i, kh, kw) = (256, 256, 3, 3)
  kern *= (1 + style[b, i])                           # input-channel modulation
  kern *= rsqrt(sum_{i,kh,kw} kern^2 + 1e-8)          # per-o L2 demodulation
  y[b] = conv2d(x[b], kern, stride 1, SAME)           # (256, 64, 64)

Mapping (data-parallel over batch; all math on device):
  - The filter bank ships host-rearranged to
      [oc(2), ic(2), fh(2), i(128), f(8), o_local(128), khw(9)]  (bf16)
    so each DMA row is 18432 contiguous elements (fat descriptors, near
    peak HBM bandwidth), o-chunk-major so conv(oc0) overlaps the oc1 DMA,
    and the mixed kernel lands directly in conv lhsT layout [i, (o,khw)]
    with no transposes. bf16 is storage precision only -- the mix
    accumulates in fp32 PSUM.
  - Mixing on TensorE: lhsT_f = w[f] * I_128 (diagonal), 16 accumulated
    bf16 matmuls per psum slice: psum[i', (o,khw)] = sum_f w_f bankT[...]
  - style modulation = per-partition scalar (1+style[i]) fused into the
    PSUM->SBUF copy.
  - demod: square + reduce-over-khw on DVE, then a ones-vector matmul
    reduces across the i partition dim; the rsqrt'd scale is applied per
    output channel when copying conv PSUM out.
  - conv: per (o_chunk, 8-row spatial tile): 18 accumulated float32r
    matmuls (i_chunk x 3 x 3) over a host-pre-padded x tile in SBUF.
"""
import sys

if "/opt/trn_rl_repo" not in sys.path:
    sys.path.insert(0, "/opt/trn_rl_repo")

import numpy as np
import concourse.bacc as bacc
import concourse.mybir as mybir
import concourse.tile as tile
from concourse.alu_op_type import AluOpType
from concourse.bass_utils import run_bass_kernel_spmd

dt = mybir.dt
AF = mybir.ActivationFunctionType

B, F, D, KK, H, W = 8, 16, 256, 3, 64, 64
HW = H * W            # 4096
KHW = KK * KK         # 9
IC = D // 128         # 2 i-chunks
OC = D // 128         # 2 o-chunks
FQ = 4                # f quarters per bank DMA tile
FPQ = F // FQ         # 4 f per quarter
OCK = 128 * KHW       # 1152 free elems per (o_chunk, khw) group
BROW = FPQ * OCK      # 4608 bf16 elems per (oc, ic, fq, i) DMA row
PW = W + 2            # padded width 66
PH_ = H + 2           # padded height 66
NS = 8                # spatial tiles (8 rows each)
SROWS = H // NS       # 8 rows per spatial tile
SN = SROWS * W        # 512 = conv matmul moving size

_COMPILED = None


def _build(num_devices=B):
    nc = bacc.Bacc("TRN2", target_bir_lowering=False, debug=False,
                   num_devices=num_devices)

    x_d = nc.dram_tensor("x", [D, PH_ * PW], dt.float32, kind="ExternalInput").ap()
    bank_d = nc.dram_tensor("bank", [OC * IC * FQ * 128, BROW], dt.bfloat16,
                            kind="ExternalInput").ap()
    breq_d = nc.dram_tensor("breq", [1, F], dt.float32, kind="ExternalInput").ap()
    sty_d = nc.dram_tensor("sty", [1, D], dt.float32, kind="ExternalInput").ap()
    # host constants: identity matrix + ones column / ones row (pure constants,
    # shipped to avoid gpsimd iota/broadcast ucode stalls at startup)
    idc_d = nc.dram_tensor("idc", [128, 129], dt.float32, kind="ExternalInput").ap()
    onesrow_d = nc.dram_tensor("onesrow", [1, 128], dt.float32, kind="ExternalInput").ap()
    y_d = nc.dram_tensor("y", [D, HW], dt.float32, kind="ExternalOutput").ap()

    f32, f32r, bf16 = dt.float32, dt.float32r, dt.bfloat16

    with tile.TileContext(nc) as tc:
        with (
            tc.tile_pool(name="setup", bufs=1) as setup,
            tc.tile_pool(name="xp", bufs=1) as xp,
            tc.tile_pool(name="bankp", bufs=3) as bankp,
            tc.tile_pool(name="kern", bufs=1) as kernp,
            tc.tile_pool(name="yout", bufs=4) as youtp,
            tc.tile_pool(name="dram", bufs=1, space="DRAM") as dramp,
            tc.tile_pool(name="mixps", bufs=1, space="PSUM") as mixps,
            tc.tile_pool(name="convps", bufs=2, space="PSUM") as convps,
            tc.tile_pool(name="normps", bufs=1, space="PSUM") as normps,
        ):
            # tiny control DMAs first so they land before the bank megabytes
            breq = setup.tile([1, F], dt.float32)
            nc.sync.dma_start(breq[:], breq_d[:])
            idc = setup.tile([128, 129], dt.float32)
            nc.sync.dma_start(idc[:], idc_d[:])
            onesrow = setup.tile([1, 128], dt.float32)
            nc.sync.dma_start(onesrow[:], onesrow_d[:])
            sty_raws = []
            for ic in range(IC):
                sc_raw = setup.tile([128, 1], dt.float32, tag=f"styraw{ic}")
                nc.sync.dma_start(
                    sc_raw[:],
                    sty_d[0:1, ic * 128:(ic + 1) * 128]
                    .rearrange("o (p u) -> (o p) u", u=1))
                sty_raws.append(sc_raw)

            # ---------- bank DMAs, in consumption order (oc0 first) ----------
            # The (oc1, ic0) block is mixed by the DVE MAC chain, which reads
            # its tiles slowly -- give those a separate slot tag and order
            # them last so they can't stall the PE block's DMAs.
            bts = {}

            def issue_bank_dmas(oc):
                if oc == 0:
                    order = [(ic, fq) for ic in range(IC) for fq in range(FQ)]
                else:
                    # interleaved so the DVE MAC chain (ic0) starts as early as
                    # possible while the PE block (ic1) still streams
                    order = [(ic, fq) for fq in range(FQ) for ic in (0, 1)]
                for ic, fq in order:
                    tag = "bankmac" if (oc, ic) == (1, 0) else "bank"
                    bt = bankp.tile([128, BROW], bf16, tag=tag)
                    row0 = ((oc * IC + ic) * FQ + fq) * 128
                    nc.sync.dma_start(bt[:], bank_d[row0:row0 + 128, :])
                    bts[(oc, ic, fq)] = bt

            issue_bank_dmas(0)

            # ---------- x: host-pre-padded, straight DMA ----------
            xpads = []
            for ic in range(IC):
                xpad = xp.tile([128, PH_ * PW], f32r, tag=f"xpad{ic}")
                nc.gpsimd.dma_start(
                    xpad[:], x_d[ic * 128:(ic + 1) * 128, :].bitcast(f32r))
                xpads.append(xpad)

            issue_bank_dmas(1)

            # ---------- setup: softmax weights, diag tiles, style columns ----------
            ident = idc[:, 0:128]

            # softmax without the max-shift: inputs are O(1), and f32 exp
            # only overflows past ~88, so the shift is dead weight on the
            # critical path to the first mix matmul.
            ex = setup.tile([1, F], f32)
            nc.scalar.activation(ex[:], breq[:], AF.Exp, bias=0.0, scale=1.0)
            sm = setup.tile([1, 1], f32)
            nc.vector.reduce_sum(sm[:], ex[:], axis=mybir.AxisListType.X)
            rs = setup.tile([1, 1], f32)
            nc.vector.reciprocal(rs[:], sm[:])
            wrow = setup.tile([1, F], f32)
            nc.vector.tensor_scalar(out=wrow[:], in0=ex[:], scalar1=rs[:],
                                    scalar2=None, op0=AluOpType.mult)
            # broadcast w across partitions with a K=1 bf16 matmul
            # (gpsimd partition_broadcast stalls ~9us on a ucode reload whose
            # fetch DMA sits behind the bank megabytes)
            wrow_b = setup.tile([1, F], bf16)
            onesrow_b = setup.tile([1, 128], bf16)
            with nc.allow_low_precision(reason="broadcast only; values tiny-rank"):
                nc.vector.tensor_copy(wrow_b[:], wrow[:])
                nc.vector.tensor_copy(onesrow_b[:], onesrow[:])
            wbps = normps.tile([128, F], f32, tag="wbc")
            nc.tensor.matmul(wbps[:], onesrow_b[:], wrow_b[:], start=True, stop=True)
            wbc = setup.tile([128, F], f32)
            nc.vector.tensor_copy(wbc[:], wbps[:])

            # per-f diagonal lhsT tiles diag(w_f), bf16 for the mix matmuls
            diags = []
            with nc.allow_low_precision(reason="bf16 diag weights; mix accumulates f32"):
                for f in range(F):
                    dg = setup.tile([128, 128], bf16, tag=f"diag{f}")
                    nc.vector.tensor_scalar(out=dg[:], in0=ident[:],
                                            scalar1=wbc[:, f:f + 1],
                                            scalar2=None, op0=AluOpType.mult)
                    diags.append(dg)

            # style columns (1 + style[i]) as per-partition scalars, per i-chunk
            stycols = []
            for ic in range(IC):
                sc = setup.tile([128, 1], f32, tag=f"sty{ic}")
                nc.scalar.activation(sc[:], sty_raws[ic][:], AF.Copy,
                                     bias=1.0, scale=1.0)
                stycols.append(sc)

            # ones column for the cross-partition (i) reduction matmul
            ones_r = setup.tile([128, 1], f32r)
            nc.vector.tensor_copy(ones_r[:], idc[:, 128:129])

            # ---------- per o_chunk: mix (both i-chunks), norm, conv ----------
            SL = ((0, 512), (512, 1024), (1024, OCK))
            km = {}       # (ic, oc) -> [128 (i), 1152 (o,khw)] f32r kernel tiles
            normcols = []
            for oc in range(OC):
                npsum = normps.tile([1, 128], f32, tag="norm")
                ics = (0, 1) if oc == 0 else (1, 0)
                for ici, ic in enumerate(ics):
                    kt = kernp.tile([128, OCK], f32r, tag=f"kern{oc}{ic}")
                    if (oc, ic) == (1, 0):
                        # DVE MAC-chain mix: frees TensorE for the conv. The
                        # chain is acc <- bank_f * w_f + acc with ping-pong
                        # accumulators (scalar_tensor_tensor has no in-place).
                        acc0 = kernp.tile([128, OCK], f32, tag="macacc0")
                        acc1 = kernp.tile([128, OCK], f32, tag="macacc1")
                        accs = (acc0, acc1)
                        with nc.allow_low_precision(reason="bf16 in, f32 acc"):
                            nc.vector.tensor_scalar(
                                out=accs[0][:], in0=bts[(oc, ic, 0)][:, 0:OCK],
                                scalar1=wbc[:, 0:1], scalar2=None,
                                op0=AluOpType.mult)
                            for f in range(1, F):
                                bt = bts[(oc, ic, f // FPQ)]
                                fo = (f % FPQ) * OCK
                                nc.vector.scalar_tensor_tensor(
                                    out=accs[f % 2][:], in0=bt[:, fo:fo + OCK],
                                    scalar=wbc[:, f:f + 1],
                                    in1=accs[(f + 1) % 2][:],
                                    op0=AluOpType.mult, op1=AluOpType.add)
                        nc.vector.tensor_scalar(
                            out=kt[:], in0=accs[(F - 1) % 2][:],
                            scalar1=stycols[ic][:], scalar2=None,
                            op0=AluOpType.mult)
                    else:
                        ps0 = mixps.tile([128, 512], f32, tag="mix0")
                        ps1 = mixps.tile([128, 512], f32, tag="mix1")
                        ps2 = mixps.tile([128, OCK - 1024], f32, tag="mix2")
                        pss = (ps0, ps1, ps2)
                        for f in range(F):
                            bt = bts[(oc, ic, f // FPQ)]
                            fo = (f % FPQ) * OCK
                            for (lo, hi), ps in zip(SL, pss):
                                nc.tensor.matmul(ps[:], diags[f][:],
                                                 bt[:, fo + lo:fo + hi],
                                                 start=(f == 0), stop=(f == F - 1))
                        # style-modulate on the way out of PSUM
                        for (lo, hi), ps in zip(SL, pss):
                            nc.vector.tensor_scalar(
                                out=kt[:, lo:hi], in0=ps[:], scalar1=stycols[ic][:],
                                scalar2=None, op0=AluOpType.mult)
                    km[(ic, oc)] = kt
                    # demod partials: square, reduce over khw, then reduce over
                    # the i partition dim with a ones-vector matmul
                    scr = kernp.tile([128, OCK], f32r, tag="sqscratch")
                    nc.vector.tensor_mul(scr[:], kt[:], kt[:])
                    redk = kernp.tile([128, 128], f32r, tag="redk")
                    with nc.allow_low_precision(reason="f32r is 4-byte"):
                        nc.vector.tensor_reduce(
                            redk[:], scr[:, :].rearrange("p (o r) -> p o r", r=KHW),
                            axis=mybir.AxisListType.X, op=AluOpType.add)
                    nc.tensor.matmul(npsum[:], ones_r[:], redk[:],
                                     start=(ici == 0), stop=(ici == IC - 1))
                # norm = 1/sqrt(npsum + eps), landed as a per-partition column
                nrow = setup.tile([1, 128], f32, tag=f"nrow{oc}")
                nc.vector.tensor_scalar_add(nrow[:], npsum[:], 1e-8)
                nsq = setup.tile([1, 128], f32, tag=f"nsq{oc}")
                nc.scalar.activation(nsq[:], nrow[:], AF.Sqrt, bias=0.0, scale=1.0)
                nrec = setup.tile([1, 128], f32, tag=f"nrec{oc}")
                nc.vector.reciprocal(nrec[:], nsq[:])
                nbounce = dramp.tile([1, 128], f32, tag=f"nb{oc}")
                nc.scalar.dma_start(nbounce[:], nrec[:])
                ncol = setup.tile([128, 1], f32, tag=f"ncol{oc}")
                nc.scalar.dma_start(ncol[:],
                                    nbounce[0:1, :].rearrange("o (p u) -> (o p) u", u=1))
                normcols.append(ncol)

                # conv: 8 spatial tiles of 8 output rows each
                # for oc1, consume ic1 (PE-mixed) before ic0 (DVE MAC chain,
                # which finishes later)
                ic_order = (0, 1) if oc == 0 else (1, 0)
                for s in range(NS):
                    r0 = s * SROWS
                    cps = convps.tile([128, SN], f32, tag="conv")
                    first = True
                    for ici, ic in enumerate(ic_order):
                        xv = xpads[ic][:, :].rearrange("p (r c) -> p r c", c=PW)
                        kv = km[(ic, oc)][:, :].rearrange("p (o r) -> p o r", r=KHW)
                        for kh in range(KK):
                            for kw in range(KK):
                                rhs = xv[:, r0 + kh: r0 + kh + SROWS, kw:kw + W]
                                nc.tensor.matmul(
                                    cps[:], kv[:, :, kh * KK + kw], rhs,
                                    start=first,
                                    stop=(ici == IC - 1 and kh == KK - 1 and kw == KK - 1))
                                first = False
                    yt = youtp.tile([128, SN], f32, tag="y")
                    nc.vector.tensor_scalar(out=yt[:], in0=cps[:],
                                            scalar1=normcols[oc][:],
                                            scalar2=None, op0=AluOpType.mult)
                    nc.gpsimd.dma_start(
                        y_d[oc * 128:(oc + 1) * 128, r0 * W:(r0 + SROWS) * W], yt[:])

    nc.compile()
    return nc


def _get_compiled():
    global _COMPILED
    if _COMPILED is None:
        _COMPILED = _build()
    return _COMPILED


def _make_in_maps(x, bank_request, style, bank_weight):
    # bank: (F, O, I, KH, KW) -> [oc, ic, fq, i, f, o_local, khw] bf16
    bf16_np = mybir.dt.np(mybir.dt.bfloat16)
    A = bank_weight.astype(np.float32).reshape(FQ, FPQ, OC, 128, IC, 128, KHW)
    #                     dims: (fq, f, oc, o_local, ic, i, khw)
    bankT = np.ascontiguousarray(A.transpose(2, 4, 0, 5, 1, 3, 6)
                                 .reshape(OC * IC * FQ * 128, BROW)).astype(bf16_np)
    idc = np.zeros((128, 129), dtype=np.float32)
    idc[:, 0:128] = np.eye(128, dtype=np.float32)
    idc[:, 128] = 1.0
    onesrow = np.ones((1, 128), dtype=np.float32)
    maps = []
    xpad = np.zeros((B, D, PH_, PW), dtype=np.float32)
    xpad[:, :, 1:1 + H, 1:1 + W] = x.astype(np.float32).reshape(B, D, H, W)
    for b in range(B):
        maps.append({
            "x": np.ascontiguousarray(xpad[b].reshape(D, PH_ * PW)),
            "bank": bankT,
            "breq": np.ascontiguousarray(
                bank_request[b].astype(np.float32).reshape(1, F)),
            "sty": np.ascontiguousarray(style[b].astype(np.float32).reshape(1, D)),
            "idc": idc,
            "onesrow": onesrow,
        })
    return maps


def run(inputs, trace=False, **trace_kwargs):
    nc = _get_compiled()
    in_maps = _make_in_maps(inputs["x"], inputs["bank_request"],
                            inputs["style"], inputs["bank_weight"])
    res = run_bass_kernel_spmd(nc, in_maps, core_ids=list(range(B)),
                               trace=trace, **trace_kwargs)
    y = np.stack([res.results[b]["y"].reshape(D, H, W) for b in range(B)], axis=0)
    return y, res


def kernel(x, bank_request, style, bank_weight):
    y, _ = run({"x": np.asarray(x), "bank_request": np.asarray(bank_request),
                "style": np.asarray(style), "bank_weight": np.asarray(bank_weight)})
    return y
